# revision 3
# baseline (speedup 1.0000x reference)
"""Bahdanau attention kernel for Trainium2 (8 NeuronCores, data-parallel over batch).

Reference computation (per batch row b):
    pq      = query @ Wq.T                       # (B, AD)
    hidden  = tanh(pq[:, None, :] + processed_memory)   # (B, T, AD)
    e       = einsum('btd,d->bt', hidden, v)     # (B, T)
    e       = where(mask, -1e30, e)
    out     = softmax(e, axis=1)

Key optimization: ~50% of positions are masked and their energies are
discarded (softmax weight exactly 0 since exp(-1e30) underflows).  The host
shard step compacts each batch row to its unmasked columns (padded to NT),
so the device streams/computes only the surviving half of processed_memory.
The host scatters device probabilities back to the full (B, T) grid with
zeros at masked positions -- bit-identical to the reference's where().

Device strategy (per core, 8 batches):
  * pm compacted+transposed to [b, db, 128, NT] fp16 so AD sits on SBUF
    partitions.  The per-d "+pq" add folds into the ScalarE tanh as a
    per-partition activation bias; tanh output hid is fp16.
  * Energies accumulate into ONE [8, NT] PSUM tile: for each 512-wide chunk,
    matmuls with a one-hot stationary VS[:, db, b, :] (column j = v iff
    j == b) add v . hid into row b and zeros into the other rows, so all
    batches share the accumulation group -- LDWEIGHTS is 8 columns (cheap)
    and every matmul streams 512 columns (efficient).  A final
    identity-stationary matmul per chunk adds a -40 penalty at padded
    positions, which drives their exp() to ~4e-18 (vs row sums >= 4e-3).
  * Softmax runs directly on the [8, NT] strip: ScalarE exp straight out of
    PSUM with fused per-partition accum_out row sums, VectorE reciprocal,
    one tensor_scalar multiply, and a dense [8, NT] DMA to DRAM.  No
    cross-partition relayout anywhere.
"""

import sys

if "/opt/trn_rl_repo" not in sys.path:
    sys.path.insert(0, "/opt/trn_rl_repo")

import numpy as np

import concourse.bacc as bacc
import concourse.bass as bass
import concourse.tile as tile
from concourse import mybir
from concourse.bass_utils import run_bass_kernel_spmd

B, T, QD, AD = 64, 4096, 1024, 256
NCORES = 8
BLOC = B // NCORES  # batches per core
KB = QD // 128      # k-blocks for the pq matmul
DB = AD // 128      # d-blocks (partition blocks of AD)
NT_DEFAULT = 2176   # device column capacity per row (>= max unmasked count)
PEN = -40.0         # energy penalty for padded columns
F32 = mybir.dt.float32
F16 = mybir.dt.float16


def build_nc(NT: int) -> bass.Bass:
    assert NT % 128 == 0
    # energy chunks must not cross PSUM bank boundaries (512 fp32)
    chunks = []
    c0 = 0
    while c0 < NT:
        cw = min(512, NT - c0)
        chunks.append((c0, cw))
        c0 += cw

    nc = bacc.Bacc(None, target_bir_lowering=False)

    pm_c = nc.declare_dram_parameter("pm_c", [BLOC, DB, 128, NT], F16, isOutput=False)
    # qT[p, kb*BLOC + b] = query[b, kb*128 + p]  (host-packed, partition-major)
    qT = nc.declare_dram_parameter("qT", [128, KB * BLOC], F16, isOutput=False)
    WqT = nc.declare_dram_parameter("WqT", [QD, AD], F16, isOutput=False)
    # one-hot v stationaries: vs[p, db, b, j] = v[db*128+p] iff j == b
    vs_d = nc.declare_dram_parameter("vs", [128, DB, BLOC, BLOC], F16, isOutput=False)
    id8_d = nc.declare_dram_parameter("id8", [BLOC, BLOC], F16, isOutput=False)
    pen_d = nc.declare_dram_parameter("pen", [BLOC, NT], F16, isOutput=False)
    out = nc.declare_dram_parameter("out", [BLOC, NT], F32, isOutput=True)

    Tanh = mybir.ActivationFunctionType.Tanh
    Exp = mybir.ActivationFunctionType.Exp

    with tile.TileContext(nc) as tc:
        with (
            tc.tile_pool(name="singles", bufs=1) as singles,
            tc.tile_pool(name="pm", bufs=6) as pm_pool,
            tc.tile_pool(name="hid", bufs=4) as hid_pool,
            tc.tile_pool(name="psum_pq", bufs=2, space="PSUM") as psum_pq,
            tc.tile_pool(name="psum_e", bufs=1, space="PSUM") as psum_e,
        ):
            # ---- constant loads on Sync (pm streams go on GpSimd) ----
            qt_sb = singles.tile([128, KB, BLOC], F16)
            nc.sync.dma_start(
                out=qt_sb, in_=qT[:, :].rearrange("p (kb b) -> p kb b", b=BLOC)
            )
            wq_sb = []
            for db in range(DB):
                w = singles.tile([128, KB, 128], F16)
                nc.sync.dma_start(
                    out=w,
                    in_=WqT[:, db * 128 : (db + 1) * 128].rearrange(
                        "(kb p) d -> p kb d", p=128
                    ),
                )
                wq_sb.append(w)
            vs_sb = singles.tile([128, DB, BLOC, BLOC], F16)
            nc.sync.dma_start(out=vs_sb, in_=vs_d[:, :, :, :])
            id8_sb = singles.tile([BLOC, BLOC], F16)
            nc.sync.dma_start(out=id8_sb, in_=id8_d[:, :])
            pen_sb = singles.tile([BLOC, NT], F16)
            nc.sync.dma_start(out=pen_sb, in_=pen_d[:, :])

            # ---- pq = Wq @ query.T, laid out [d % 128, dblk, b] ----
            pq_sb = singles.tile([128, DB, BLOC], F32)
            for db in range(DB):
                ppq = psum_pq.tile([128, BLOC], F32, tag="pq")
                for k in range(KB):
                    nc.tensor.matmul(
                        ppq,
                        lhsT=wq_sb[db][:, k, :],
                        rhs=qt_sb[:, k, :],
                        start=(k == 0),
                        stop=(k == KB - 1),
                    )
                nc.vector.tensor_copy(out=pq_sb[:, db, :], in_=ppq)

            # ---- energies accumulator [8, NT] in PSUM ----
            ep = psum_e.tile([BLOC, NT], F32, tag="e")

            # ---- main loop: tanh, then one-hot v matmuls per chunk ----
            for b in range(BLOC):
                hid = []
                for db in range(DB):
                    pm_sb = pm_pool.tile([128, NT], F16)
                    nc.gpsimd.dma_start(out=pm_sb, in_=pm_c[b, db])
                    h = hid_pool.tile([128, NT], F16)
                    nc.scalar.activation(
                        out=h,
                        in_=pm_sb,
                        func=Tanh,
                        bias=pq_sb[:, db, b : b + 1],
                        scale=1.0,
                    )
                    hid.append(h)
                for c0, cw in chunks:
                    for db in range(DB):
                        nc.tensor.matmul(
                            ep[:, c0 : c0 + cw],
                            lhsT=vs_sb[:, db, b, :],
                            rhs=hid[db][:, c0 : c0 + cw],
                            start=(b == 0 and db == 0),
                            stop=False,
                        )
                    if b == BLOC - 1:
                        # kill padded positions: e += -40 there
                        nc.tensor.matmul(
                            ep[:, c0 : c0 + cw],
                            lhsT=id8_sb,
                            rhs=pen_sb[:, c0 : c0 + cw],
                            start=False,
                            stop=True,
                        )

            # ---- masked softmax on the [8, NT] strip ----
            ex_sb = singles.tile([BLOC, NT], F32)
            rowsum = singles.tile([BLOC, 1], F32)
            rinv_sb = singles.tile([BLOC, 1], F32)
            nc.scalar.activation(out=ex_sb, in_=ep, func=Exp, accum_out=rowsum)
            nc.vector.reciprocal(out=rinv_sb, in_=rowsum)
            nc.vector.tensor_scalar_mul(out=ex_sb, in0=ex_sb, scalar1=rinv_sb)
            nc.sync.dma_start(out=out[:, :], in_=ex_sb)

    nc.finalize()
    return nc


_CACHE: dict = {}


def _get_nc(NT: int) -> bass.Bass:
    if NT not in _CACHE:
        _CACHE[NT] = build_nc(NT)
    return _CACHE[NT]


def make_in_maps(query, processed_memory, mask, Wq, v):
    query = np.asarray(query, dtype=np.float32)
    pm = np.asarray(processed_memory, dtype=np.float32)
    mask_b = np.asarray(mask).astype(bool)
    Wq = np.asarray(Wq, dtype=np.float32)
    v = np.asarray(v, dtype=np.float32)

    idx_all = []
    n_all = []
    for gb in range(B):
        idx = np.flatnonzero(~mask_b[gb])
        idx_all.append(idx)
        n_all.append(len(idx))
    n_max = max(n_all)
    NT = max(NT_DEFAULT, ((n_max + 127) // 128) * 128)

    WqT = np.ascontiguousarray(Wq.T).astype(np.float16)  # (QD, AD)
    vs = np.zeros((128, DB, BLOC, BLOC), dtype=np.float16)
    for db in range(DB):
        for b in range(BLOC):
            vs[:, db, b, b] = v[db * 128 : (db + 1) * 128]
    id8 = np.eye(BLOC, dtype=np.float16)

    in_maps = []
    for i in range(NCORES):
        sl = slice(i * BLOC, (i + 1) * BLOC)
        pm_core = np.empty((BLOC, DB, 128, NT), dtype=np.float16)
        pen = np.zeros((BLOC, NT), dtype=np.float16)
        for b in range(BLOC):
            gb = i * BLOC + b
            idx = idx_all[gb]
            n = n_all[gb]
            if n == 0:
                idx_pad = np.zeros(NT, dtype=np.int64)
            elif n < NT:
                idx_pad = np.concatenate(
                    [idx, np.full(NT - n, idx[-1], dtype=idx.dtype)]
                )
            else:
                idx_pad = idx
            # [NT, AD] -> [AD, NT] -> [DB, 128, NT]
            pmt = pm[gb][idx_pad].T.astype(np.float16)
            pm_core[b] = pmt.reshape(DB, 128, NT)
            pen[b, n:] = PEN
        in_maps.append(
            {
                "pm_c": pm_core,
                "qT": np.ascontiguousarray(
                    query[sl]
                    .T.reshape(KB, 128, BLOC)
                    .transpose(1, 0, 2)
                    .reshape(128, KB * BLOC)
                ).astype(np.float16),
                "WqT": WqT,
                "vs": vs,
                "id8": id8,
                "pen": pen,
            }
        )
    return in_maps, idx_all, n_all, NT


def run_spmd(in_maps, NT=NT_DEFAULT, **kwargs):
    return run_bass_kernel_spmd(_get_nc(NT), in_maps, list(range(NCORES)), **kwargs)


def kernel(query, processed_memory, mask, Wq, v) -> np.ndarray:
    in_maps, idx_all, n_all, NT = make_in_maps(query, processed_memory, mask, Wq, v)
    res = run_spmd(in_maps, NT=NT)
    out_full = np.zeros((B, T), dtype=np.float32)
    for i in range(NCORES):
        oc = res.results[i]["out"]
        for b in range(BLOC):
            gb = i * BLOC + b
            n = n_all[gb]
            if n == 0:
                # reference: all energies equal (-1e30) -> uniform softmax
                out_full[gb, :] = 1.0 / T
            else:
                out_full[gb, idx_all[gb]] = oc[b, :n]
    return out_full


# revision 5
# speedup vs baseline: 1.2755x; 1.2755x over previous
"""Bahdanau attention kernel for Trainium2 (8 NeuronCores, data-parallel over batch).

Reference computation (per batch row b):
    pq      = query @ Wq.T                       # (B, AD)
    hidden  = tanh(pq[:, None, :] + processed_memory)   # (B, T, AD)
    e       = einsum('btd,d->bt', hidden, v)     # (B, T)
    e       = where(mask, -1e30, e)
    out     = softmax(e, axis=1)

Key optimization: ~50% of positions are masked and their energies are
discarded (softmax weight exactly 0 since exp(-1e30) underflows).  The host
shard step compacts each batch row to its unmasked columns (padded to NT),
so the device streams/computes only the surviving half of processed_memory.
The host scatters device probabilities back to the full (B, T) grid with
zeros at masked positions -- bit-identical to the reference's where().

Device strategy (per core, 8 batches):
  * pm compacted+transposed to [b, db, 128, NT] fp16 so AD sits on SBUF
    partitions.  The per-d "+pq" add folds into the ScalarE tanh as a
    per-partition activation bias; tanh output hid is fp16.
  * Energies accumulate into ONE [8, NT] PSUM tile: for each 512-wide chunk,
    matmuls with a one-hot stationary VS[:, db, b, :] (column j = v iff
    j == b) add v . hid into row b and zeros into the other rows, so all
    batches share the accumulation group -- LDWEIGHTS is 8 columns (cheap)
    and every matmul streams 512 columns (efficient).  A final
    identity-stationary matmul per chunk adds a -40 penalty at padded
    positions, which drives their exp() to ~4e-18 (vs row sums >= 4e-3).
  * Softmax runs directly on the [8, NT] strip: ScalarE exp straight out of
    PSUM with fused per-partition accum_out row sums, VectorE reciprocal,
    one tensor_scalar multiply, and a dense [8, NT] DMA to DRAM.  No
    cross-partition relayout anywhere.
"""

import sys

if "/opt/trn_rl_repo" not in sys.path:
    sys.path.insert(0, "/opt/trn_rl_repo")

import numpy as np

import concourse.bacc as bacc
import concourse.bass as bass
import concourse.tile as tile
from concourse import mybir
from concourse.bass_utils import run_bass_kernel_spmd

B, T, QD, AD = 64, 4096, 1024, 256
NCORES = 8
BLOC = B // NCORES  # batches per core
KB = QD // 128      # k-blocks for the pq matmul
DB = AD // 128      # d-blocks (partition blocks of AD)
NT_DEFAULT = 2176   # device column capacity per row (>= max unmasked count)
PEN = -40.0         # energy penalty for padded columns
F32 = mybir.dt.float32
F16 = mybir.dt.float16


def build_nc(NT: int) -> bass.Bass:
    assert NT % 128 == 0
    # energy chunks must not cross PSUM bank boundaries (512 fp32)
    chunks = []
    c0 = 0
    while c0 < NT:
        cw = min(512, NT - c0)
        chunks.append((c0, cw))
        c0 += cw

    nc = bacc.Bacc(None, target_bir_lowering=False)

    pm_c = nc.declare_dram_parameter("pm_c", [BLOC, DB, 128, NT], F16, isOutput=False)
    # qT[p, kb*BLOC + b] = query[b, kb*128 + p]  (host-packed, partition-major)
    qT = nc.declare_dram_parameter("qT", [128, KB * BLOC], F16, isOutput=False)
    WqT = nc.declare_dram_parameter("WqT", [QD, AD], F16, isOutput=False)
    # one-hot v stationaries: vs[p, db, b, j] = v[db*128+p] iff j == b
    vs_d = nc.declare_dram_parameter("vs", [128, DB, BLOC, BLOC], F16, isOutput=False)
    id8_d = nc.declare_dram_parameter("id8", [BLOC, BLOC], F16, isOutput=False)
    pen_d = nc.declare_dram_parameter("pen", [BLOC, NT], F16, isOutput=False)
    out = nc.declare_dram_parameter("out", [BLOC, NT], F32, isOutput=True)

    Tanh = mybir.ActivationFunctionType.Tanh
    Exp = mybir.ActivationFunctionType.Exp

    with tile.TileContext(nc) as tc:
        with (
            tc.tile_pool(name="singles", bufs=1) as singles,
            tc.tile_pool(name="pm", bufs=6) as pm_pool,
            tc.tile_pool(name="hid", bufs=4) as hid_pool,
            tc.tile_pool(name="psum_pq", bufs=2, space="PSUM") as psum_pq,
            tc.tile_pool(name="psum_e", bufs=1, space="PSUM") as psum_e,
        ):
            # ---- leading loads, ordered by when they gate compute:
            # wq/qt gate pq; pm[0] gates the first tanh; the rest trail.
            qt_sb = singles.tile([128, KB, BLOC], F16)
            nc.sync.dma_start(
                out=qt_sb, in_=qT[:, :].rearrange("p (kb b) -> p kb b", b=BLOC)
            )
            wq_sb = []
            for db in range(DB):
                w = singles.tile([128, KB, 128], F16)
                nc.sync.dma_start(
                    out=w,
                    in_=WqT[:, db * 128 : (db + 1) * 128].rearrange(
                        "(kb p) d -> p kb d", p=128
                    ),
                )
                wq_sb.append(w)
            pm_tiles = {}
            for b in range(2):
                for db in range(DB):
                    pm_sb = pm_pool.tile([128, NT], F16)
                    nc.sync.dma_start(out=pm_sb, in_=pm_c[b, db])
                    pm_tiles[(b, db)] = pm_sb
            vs_sb = singles.tile([128, DB, BLOC, BLOC], F16)
            nc.sync.dma_start(out=vs_sb, in_=vs_d[:, :, :, :])
            id8_sb = singles.tile([BLOC, BLOC], F16)
            nc.sync.dma_start(out=id8_sb, in_=id8_d[:, :])
            pen_sb = singles.tile([BLOC, NT], F16)
            nc.sync.dma_start(out=pen_sb, in_=pen_d[:, :])

            # ---- pq = Wq @ query.T, laid out [d % 128, dblk, b] ----
            pq_sb = singles.tile([128, DB, BLOC], F32)
            for db in range(DB):
                ppq = psum_pq.tile([128, BLOC], F32, tag="pq")
                for k in range(KB):
                    nc.tensor.matmul(
                        ppq,
                        lhsT=wq_sb[db][:, k, :],
                        rhs=qt_sb[:, k, :],
                        start=(k == 0),
                        stop=(k == KB - 1),
                    )
                nc.vector.tensor_copy(out=pq_sb[:, db, :], in_=ppq)

            # ---- energies accumulator [8, NT] in PSUM ----
            ep = psum_e.tile([BLOC, NT], F32, tag="e")

            # ---- main loop: per (b, db): tanh then its matmuls right away
            # (short PE gaps keep the HAM clock gate warm) ----
            for b in range(BLOC):
                for db in range(DB):
                    if (b, db) in pm_tiles:
                        pm_sb = pm_tiles[(b, db)]
                    else:
                        pm_sb = pm_pool.tile([128, NT], F16)
                        nc.sync.dma_start(out=pm_sb, in_=pm_c[b, db])
                    h = hid_pool.tile([128, NT], F16)
                    nc.scalar.activation(
                        out=h,
                        in_=pm_sb,
                        func=Tanh,
                        bias=pq_sb[:, db, b : b + 1],
                        scale=1.0,
                    )
                    for c0, cw in chunks:
                        nc.tensor.matmul(
                            ep[:, c0 : c0 + cw],
                            lhsT=vs_sb[:, db, b, :],
                            rhs=h[:, c0 : c0 + cw],
                            start=(b == 0 and db == 0),
                            stop=False,
                        )
                if b == BLOC - 1:
                    # kill padded positions: e += -40 there
                    for c0, cw in chunks:
                        nc.tensor.matmul(
                            ep[:, c0 : c0 + cw],
                            lhsT=id8_sb,
                            rhs=pen_sb[:, c0 : c0 + cw],
                            start=False,
                            stop=True,
                        )

            # ---- masked softmax on the [8, NT] strip ----
            ex_sb = singles.tile([BLOC, NT], F32)
            rowsum = singles.tile([BLOC, 1], F32)
            rinv_sb = singles.tile([BLOC, 1], F32)
            nc.scalar.activation(out=ex_sb, in_=ep, func=Exp, accum_out=rowsum)
            nc.vector.reciprocal(out=rinv_sb, in_=rowsum)
            nc.vector.tensor_scalar_mul(out=ex_sb, in0=ex_sb, scalar1=rinv_sb)
            nc.sync.dma_start(out=out[:, :], in_=ex_sb)

    nc.finalize()
    return nc


_CACHE: dict = {}


def _get_nc(NT: int) -> bass.Bass:
    if NT not in _CACHE:
        _CACHE[NT] = build_nc(NT)
    return _CACHE[NT]


def make_in_maps(query, processed_memory, mask, Wq, v):
    query = np.asarray(query, dtype=np.float32)
    pm = np.asarray(processed_memory, dtype=np.float32)
    mask_b = np.asarray(mask).astype(bool)
    Wq = np.asarray(Wq, dtype=np.float32)
    v = np.asarray(v, dtype=np.float32)

    idx_all = []
    n_all = []
    for gb in range(B):
        idx = np.flatnonzero(~mask_b[gb])
        idx_all.append(idx)
        n_all.append(len(idx))
    n_max = max(n_all)
    NT = max(NT_DEFAULT, ((n_max + 127) // 128) * 128)

    WqT = np.ascontiguousarray(Wq.T).astype(np.float16)  # (QD, AD)
    vs = np.zeros((128, DB, BLOC, BLOC), dtype=np.float16)
    for db in range(DB):
        for b in range(BLOC):
            vs[:, db, b, b] = v[db * 128 : (db + 1) * 128]
    id8 = np.eye(BLOC, dtype=np.float16)

    in_maps = []
    for i in range(NCORES):
        sl = slice(i * BLOC, (i + 1) * BLOC)
        pm_core = np.empty((BLOC, DB, 128, NT), dtype=np.float16)
        pen = np.zeros((BLOC, NT), dtype=np.float16)
        for b in range(BLOC):
            gb = i * BLOC + b
            idx = idx_all[gb]
            n = n_all[gb]
            if n == 0:
                idx_pad = np.zeros(NT, dtype=np.int64)
            elif n < NT:
                idx_pad = np.concatenate(
                    [idx, np.full(NT - n, idx[-1], dtype=idx.dtype)]
                )
            else:
                idx_pad = idx
            # [NT, AD] -> [AD, NT] -> [DB, 128, NT]
            pmt = pm[gb][idx_pad].T.astype(np.float16)
            pm_core[b] = pmt.reshape(DB, 128, NT)
            pen[b, n:] = PEN
        in_maps.append(
            {
                "pm_c": pm_core,
                "qT": np.ascontiguousarray(
                    query[sl]
                    .T.reshape(KB, 128, BLOC)
                    .transpose(1, 0, 2)
                    .reshape(128, KB * BLOC)
                ).astype(np.float16),
                "WqT": WqT,
                "vs": vs,
                "id8": id8,
                "pen": pen,
            }
        )
    return in_maps, idx_all, n_all, NT


def run_spmd(in_maps, NT=NT_DEFAULT, **kwargs):
    return run_bass_kernel_spmd(_get_nc(NT), in_maps, list(range(NCORES)), **kwargs)


def kernel(query, processed_memory, mask, Wq, v) -> np.ndarray:
    in_maps, idx_all, n_all, NT = make_in_maps(query, processed_memory, mask, Wq, v)
    res = run_spmd(in_maps, NT=NT)
    out_full = np.zeros((B, T), dtype=np.float32)
    for i in range(NCORES):
        oc = res.results[i]["out"]
        for b in range(BLOC):
            gb = i * BLOC + b
            n = n_all[gb]
            if n == 0:
                # reference: all energies equal (-1e30) -> uniform softmax
                out_full[gb, :] = 1.0 / T
            else:
                out_full[gb, idx_all[gb]] = oc[b, :n]
    return out_full
